# revision 1
# baseline (speedup 1.0000x reference)
"""C3D-style circulant-block 3D CNN forward pass on 8 Trainium2 NeuronCores.

Sharding: data-parallel over batch (8 samples -> 8 cores). Training-mode
BatchNorm batch statistics are combined across cores with tiny per-layer
f32 AllReduces of (mean, E[x^2]) per channel, split so that the stats of
the first output chunks reduce while the last chunk still computes.

Weights for conv3b..conv5b are uploaded in block-circulant compressed
form: with channels relabeled c' = (c%4)*(C/4) + c//4, every dense
128x128 (m-chunk, k-chunk, tap) weight block is a plain slice of a small
packed tensor, so the 4x (or 2x) expansion never materializes in HBM.

Device kernel per core (per sample):
  conv1 via host-side im2col (K=81) -> matmul stream, pool+stats straight
  from PSUM. conv2..conv5b as shift-and-accumulate implicit GEMM with
  input channels on partitions, 27 taps accumulated in PSUM (N<=448).
  Per conv tile: DVE bn_stats on PSUM, DVE tensor_max maxpool (valid
  before the BN affine because the BN scale g*rsqrt(var+eps) is >= 0).
  Stats AllReduce -> fused BN+ReLU via ACT into the next padded input.
  Tail: special-padded pool5 -> global mean (1/16 folded into FC
  weights) -> FC matmul -> logits.
"""

import numpy as np
import ml_dtypes

import concourse.bass as bass
import concourse.mybir as mybir
import concourse.tile as tile
from concourse import bacc
from concourse.bass_utils import run_bass_kernel_spmd

F32 = mybir.dt.float32
BF16 = mybir.dt.bfloat16
NPBF16 = ml_dtypes.bfloat16
RELU = mybir.ActivationFunctionType.Relu
COPY = mybir.ActivationFunctionType.Copy
SQRT = mybir.ActivationFunctionType.Sqrt
IDENT = mybir.ActivationFunctionType.Identity
ADD = mybir.AluOpType.add
MAX = mybir.AluOpType.max
AXX = mybir.AxisListType.X
EPS = 1e-5
N_CORES = 8

TAPS = [(kd, kh, kw) for kd in range(3) for kh in range(3) for kw in range(3)]

# name, Cin, Cout, D, H, W, pooled  (pooled=None -> raw stage for pool5)
GEN_LAYERS = [
    ("3a", 128, 256, 8, 28, 28, False),
    ("3b", 256, 256, 8, 28, 28, True),
    ("4a", 256, 512, 4, 14, 14, False),
    ("4b", 512, 512, 4, 14, 14, True),
    ("5a", 512, 512, 2, 7, 7, False),
    ("5b", 512, 512, 2, 7, 7, None),
]


def layer_tiles(D, H, W):
    """Output tiles (z0, y0, dz, dy) with dz*dy*W <= 448 columns."""
    if H == 28:          # conv3: 2 z-tiles x 7 y-tiles of [4z,4y,28x]=448
        return [(z0, y0, 4, 4) for z0 in (0, 4) for y0 in range(0, 28, 4)]
    if H == 14:          # conv4: 2 tiles of [2z,14y,14x]=392
        return [(z0, 0, 2, 14) for z0 in (0, 2)]
    return [(0, 0, 2, 7)]  # conv5: 1 tile of [2z,7y,7x]=98


def relab(C):
    """Channel relabeling perm: pos[x] = (x%4)*(C//4) + x//4."""
    x = np.arange(C)
    return (x % 4) * (C // 4) + x // 4


def apply_perm(v, pos):
    out = np.empty_like(v)
    out[pos] = v
    return out


def circ_expand_np(c):
    c = np.asarray(c, np.float32)
    P, Q, b = c.shape[0], c.shape[1], c.shape[2]
    r = np.arange(b)
    idx = (r[:, None] - r[None, :]) % b
    w = c[:, :, idx]  # (P, Q, b, b, k, k, k)
    w = np.transpose(w, (0, 2, 1, 3, 4, 5, 6))
    return w.reshape(P * b, Q * b, *c.shape[3:])


def pack_comp(cw, Cin, Cout):
    """Compressed circulant pack -> [128, 27, J, 128] bf16.

    cw: raw block-circulant params (Cout//4, Cin//4, 4, 3,3,3).
    Dense block for (m-chunk m, k-chunk c, tap) is st[:, tap, j(m,c), :].
    """
    cb = np.asarray(cw, np.float32).reshape(Cout // 4, Cin // 4, 4, 27)
    P_ = np.arange(128)[None, :]          # output col within chunk
    p = np.arange(128)[:, None]           # partition
    if Cin == 512 and Cout == 512:
        # st[q, tap, k, P] = cb[P, q, k];  j = (m-c)%4
        st = cb[P_, p, :, :]              # [128q, 128P, 4k, 27] -> reorder
        st = np.transpose(st, (0, 3, 2, 1))
        J = 4
    elif Cin == 256 and Cout == 512:
        # st[p, tap, j, P] = cb[P, p%64, (j-p//64)%4];  j = (m-2c)%4
        J = 4
        st = np.empty((128, 27, 4, 128), np.float32)
        for j in range(4):
            k = (j - p // 64) % 4         # [128,1]
            st[:, :, j, :] = np.transpose(
                cb[P_, p % 64, k, :], (0, 2, 1))
    elif Cin == 256 and Cout == 256:
        # st[p, tap, j, rr*64+P] = cb[P, p%64, (2j+rr-p//64)%4]; j=(m-c)%2
        J = 2
        st = np.empty((128, 27, 2, 128), np.float32)
        Pq = np.arange(64)[None, :]
        for j in range(2):
            for rr in range(2):
                k = (2 * j + rr - p // 64) % 4
                st[:, :, j, rr * 64:rr * 64 + 64] = np.transpose(
                    cb[Pq, p % 64, k, :], (0, 2, 1))
    else:
        raise ValueError((Cin, Cout))
    return np.ascontiguousarray(st, dtype=NPBF16), J


def pack_dense(wd, Kch, Mch, pin=None, pout=None):
    """Dense pack -> [128, Kch, Mch, 27, 128] bf16 with optional perms."""
    Co, Ci = wd.shape[0], wd.shape[1]
    wt = np.asarray(wd, np.float32).reshape(Co, Ci, 27)
    if pout is not None:
        wt = wt[np.argsort(pout)]         # row co' = orig channel argsort
    if pin is not None:
        wt = wt[:, np.argsort(pin)]
    wt = wt.transpose(1, 2, 0)            # (Ci', 27, Co')
    wt = wt.reshape(Kch, 128, 27, Mch, 128)
    wt = wt.transpose(1, 0, 3, 2, 4)      # [128, Kch, Mch, 27, 128]
    return np.ascontiguousarray(wt, dtype=NPBF16)


def host_prep(inputs):
    g = {k: np.asarray(v, np.float32) for k, v in inputs.items()}
    shared = {}
    perm256 = relab(256)
    perm512 = relab(512)

    # conv1 (natural channel order)
    w1 = g["conv1_w"]
    shared["w1"] = np.ascontiguousarray(
        w1.transpose(1, 2, 3, 4, 0).reshape(81, 64), dtype=NPBF16)
    # conv2 (kd-packed, natural in/out)
    w2 = circ_expand_np(g["c2"])  # (128, 64, 3,3,3)
    w2t = w2.transpose(2, 1, 3, 4, 0)  # (kd, ci, kh, kw, co)
    shared["w2a"] = np.ascontiguousarray(
        w2t[0:2].reshape(128, 9, 128), dtype=NPBF16)
    shared["w2b"] = np.ascontiguousarray(
        w2t[2].reshape(64, 9, 128), dtype=NPBF16)
    # conv3a dense: natural in, perm256 out
    w3a = circ_expand_np(g["c3a"]).reshape(256, 128, 27)
    shared["w3a"] = pack_dense(w3a, 1, 2, pin=None, pout=perm256)
    # compressed layers
    shared["w3b"], _ = pack_comp(g["c3b"], 256, 256)
    shared["w4a"], _ = pack_comp(g["c4a"], 256, 512)
    shared["w4b"], _ = pack_comp(g["c4b"], 512, 512)
    shared["w5a"], _ = pack_comp(g["c5a"], 512, 512)
    shared["w5b"], _ = pack_comp(g["c5b"], 512, 512)

    # bn params (relabeled where the layer's output channels are permuted)
    def pk(v, parts):
        v = np.asarray(v, np.float32)
        mch = v.size // parts
        return np.ascontiguousarray(v.reshape(mch, parts).T)
    perms = {"1": None, "2": None, "3a": perm256, "3b": perm256,
             "4a": perm512, "4b": perm512, "5a": perm512, "5b": perm512}
    parts = {"1": 64, "2": 128, "3a": 128, "3b": 128, "4a": 128,
             "4b": 128, "5a": 128, "5b": 128}
    for name in ("1", "2", "3a", "3b", "4a", "4b", "5a", "5b"):
        gv, bv = g[f"g{name}"], g[f"b{name}"]
        assert np.all(gv >= 0), "pool/BN commute needs g >= 0"
        if perms[name] is not None:
            gv = apply_perm(gv, perms[name])
            bv = apply_perm(bv, perms[name])
        shared[f"gn{name}"] = pk(gv, parts[name])
        shared[f"bn{name}"] = pk(bv, parts[name])
    # fc (fold /16 global-mean into weights; relabel input features)
    fcw = (g["fc_w"].T / 16.0)  # (512, 101)
    fcw = fcw[np.argsort(perm512)]
    shared["fcw"] = np.ascontiguousarray(
        fcw.reshape(4, 128, 101).transpose(1, 0, 2), dtype=NPBF16)
    shared["fcb"] = np.ascontiguousarray(g["fc_b"].reshape(101, 1))
    # per-core conv1 im2col
    x = g["x"]  # (8, 3, 16, 112, 112)
    x1_list = []
    for i in range(x.shape[0]):
        xp = np.zeros((3, 18, 114, 114), np.float32)
        xp[:, 1:17, 1:113, 1:113] = x[i]
        sw = np.lib.stride_tricks.sliding_window_view(xp, (3, 3, 3),
                                                      axis=(1, 2, 3))
        b1 = sw.transpose(0, 4, 5, 6, 1, 2, 3).reshape(81, 16, 12544)
        x1_list.append(np.ascontiguousarray(b1, dtype=NPBF16))
    return shared, x1_list


def build_bass(n_cores, fake_cc=False):
    nc = bacc.Bacc("TRN2", target_bir_lowering=False, debug=False,
                   num_devices=n_cores)
    rg = [list(range(n_cores))]

    din = {}
    din["x1"] = nc.dram_tensor("x1", [81, 16, 12544], BF16,
                               kind="ExternalInput")
    din["w1"] = nc.dram_tensor("w1", [81, 64], BF16, kind="ExternalInput")
    din["w2a"] = nc.dram_tensor("w2a", [128, 9, 128], BF16,
                                kind="ExternalInput")
    din["w2b"] = nc.dram_tensor("w2b", [64, 9, 128], BF16,
                                kind="ExternalInput")
    din["w3a"] = nc.dram_tensor("w3a", [128, 1, 2, 27, 128], BF16,
                                kind="ExternalInput")
    din["w3b"] = nc.dram_tensor("w3b", [128, 27, 2, 128], BF16,
                                kind="ExternalInput")
    for name in ("4a", "4b", "5a", "5b"):
        din[f"w{name}"] = nc.dram_tensor(f"w{name}", [128, 27, 4, 128],
                                         BF16, kind="ExternalInput")
    din["gn1"] = nc.dram_tensor("gn1", [64, 1], F32, kind="ExternalInput")
    din["bn1"] = nc.dram_tensor("bn1", [64, 1], F32, kind="ExternalInput")
    for name, c in [("2", 128), ("3a", 256), ("3b", 256), ("4a", 512),
                    ("4b", 512), ("5a", 512), ("5b", 512)]:
        mch = c // 128
        din[f"gn{name}"] = nc.dram_tensor(f"gn{name}", [128, mch], F32,
                                          kind="ExternalInput")
        din[f"bn{name}"] = nc.dram_tensor(f"bn{name}", [128, mch], F32,
                                          kind="ExternalInput")
    din["fcw"] = nc.dram_tensor("fcw", [128, 4, 101], BF16,
                                kind="ExternalInput")
    din["fcb"] = nc.dram_tensor("fcb", [101, 1], F32, kind="ExternalInput")
    logits = nc.dram_tensor("logits", [101, 1], F32, kind="ExternalOutput")

    with tile.TileContext(nc) as tc:
        build_graph(tc, din, logits, rg, fake_cc)
    nc.compile()
    return nc


def build_graph(tc, din, logits, rg, fake_cc=False):
    nc = tc.nc
    import contextlib
    ctx = contextlib.ExitStack()
    with ctx:
        singles = ctx.enter_context(tc.tile_pool(name="singles", bufs=1))
        small = ctx.enter_context(tc.tile_pool(name="small", bufs=2))
        statsp = ctx.enter_context(tc.tile_pool(name="statsp", bufs=2))
        psum = ctx.enter_context(tc.tile_pool(name="psum", bufs=3,
                                              space="PSUM"))
        psfc = ctx.enter_context(tc.tile_pool(name="psfc", bufs=1,
                                              space="PSUM"))
        pwp = ctx.enter_context(tc.tile_pool(name="pwp", bufs=4))
        arenaA = ctx.enter_context(tc.tile_pool(name="arenaA", bufs=1))
        arenaB = ctx.enter_context(tc.tile_pool(name="arenaB", bufs=1))
        wpool = ctx.enter_context(tc.tile_pool(name="wpool", bufs=2))
        dram = ctx.enter_context(tc.tile_pool(name="dram", bufs=1,
                                              space="DRAM"))

        eps_t = singles.tile([128, 1], F32, tag="eps")
        nc.vector.memset(eps_t[:], EPS)

        # persistent small params (scalar queue)
        params = {}
        for name, parts in [("1", 64), ("2", 128), ("3a", 128), ("3b", 128),
                            ("4a", 128), ("4b", 128), ("5a", 128),
                            ("5b", 128)]:
            mch = din[f"gn{name}"].shape[1]
            gt = singles.tile([parts, mch], F32, tag=f"g{name}")
            bt = singles.tile([parts, mch], F32, tag=f"b{name}")
            nc.scalar.dma_start(gt[:], din[f"gn{name}"][:])
            nc.scalar.dma_start(bt[:], din[f"bn{name}"][:])
            params[name] = (gt, bt)

        w1_sb = singles.tile([81, 64], BF16, tag="w1")
        nc.scalar.dma_start(w1_sb[:], din["w1"][:])
        w2a_sb = singles.tile([128, 9, 128], BF16, tag="w2a")
        nc.scalar.dma_start(w2a_sb[:], din["w2a"][:])
        w2b_sb = singles.tile([64, 9, 128], BF16, tag="w2b")
        nc.scalar.dma_start(w2b_sb[:], din["w2b"][:])
        fcw_sb = singles.tile([128, 4, 101], BF16, tag="fcw")
        nc.scalar.dma_start(fcw_sb[:], din["fcw"][:])
        fcb_sb = singles.tile([101, 1], F32, tag="fcb")
        nc.scalar.dma_start(fcb_sb[:], din["fcb"][:])

        # warm up the collective path (overlaps conv1)
        for wi in range(2):
            wdin = dram.tile([64, 2], F32, tag=f"warm{wi}")
            wdout = dram.tile([64, 2], F32, tag=f"warmo{wi}",
                              addr_space="Shared")
            if fake_cc:
                nc.scalar.dma_start(wdout[:], wdin[:])
            else:
                nc.gpsimd.collective_compute(
                    "AllReduce", ADD, replica_groups=rg,
                    ins=[wdin.opt()], outs=[wdout.opt()])

        y1_dram = dram.tile([64, 16, 3136], BF16, tag="y1d")

        ar_seq = [0]

        def bn_reduce_part(name, stats_t, parts, m_list, Tn):
            """Reduce stats for chunks in m_list -> (s, t) [parts, len]."""
            nm = len(m_list)
            mv = small.tile([parts, nm, 2], F32, tag="mv")
            for i, m in enumerate(m_list):
                nc.vector.bn_aggr(mv[:, i], stats_t[:, m])
            cc = small.tile([parts, nm, 2], F32, tag="cc")
            sq = small.tile([parts, nm], F32, tag="sq")
            nc.vector.tensor_mul(sq[:], mv[:, :, 0], mv[:, :, 0])
            nc.vector.tensor_add(cc[:, :, 1], mv[:, :, 1], sq[:])
            nc.vector.tensor_copy(cc[:, :, 0], mv[:, :, 0])
            aid = ar_seq[0]
            ar_seq[0] += 1
            ccin = dram.tile([parts, nm * 2], F32, tag=f"ccin{aid}")
            ccout = dram.tile([parts, nm * 2], F32, tag=f"ccout{aid}",
                              addr_space="Shared")
            nc.scalar.dma_start(ccin[:],
                                cc[:].rearrange("p m two -> p (m two)"))
            if fake_cc:
                nc.scalar.dma_start(ccout[:], ccin[:])
            else:
                nc.gpsimd.collective_compute(
                    "AllReduce", ADD, replica_groups=rg,
                    ins=[ccin.opt()], outs=[ccout.opt()])
            ar = small.tile([parts, nm, 2], F32, tag="ar")
            nc.scalar.dma_start(ar[:].rearrange("p m two -> p (m two)"),
                                ccout[:])
            inv_n = 1.0 / len(rg[0])
            mg = small.tile([parts, nm], F32, tag="mg")
            e2 = small.tile([parts, nm], F32, tag="e2")
            nc.vector.tensor_scalar_mul(mg[:], ar[:, :, 0], inv_n)
            nc.vector.tensor_scalar_mul(e2[:], ar[:, :, 1], inv_n)
            sq2 = small.tile([parts, nm], F32, tag="sq2")
            nc.vector.tensor_mul(sq2[:], mg[:], mg[:])
            varg = small.tile([parts, nm], F32, tag="varg")
            nc.vector.tensor_sub(varg[:], e2[:], sq2[:])
            sd = small.tile([parts, nm], F32, tag="sd")
            nc.scalar.activation(sd[:], varg[:], SQRT, bias=eps_t[:parts])
            inv = small.tile([parts, nm], F32, tag="inv")
            nc.vector.reciprocal(inv[:], sd[:])
            gt, bt = params[name]
            s_t = small.tile([parts, nm], F32, tag=f"s{aid}")
            t_t = small.tile([parts, nm], F32, tag=f"t{aid}")
            gsel = gt[:, m_list[0]:m_list[0] + nm]
            bsel = bt[:, m_list[0]:m_list[0] + nm]
            nc.vector.tensor_mul(s_t[:], inv[:], gsel)
            tmn = small.tile([parts, nm], F32, tag="tmn")
            nc.vector.tensor_mul(tmn[:], mg[:], s_t[:])
            nc.vector.tensor_sub(t_t[:], bsel, tmn[:])
            return s_t, t_t

        def zero_borders_m(P, m, Dp, Hp, Wp):
            nc.vector.memset(P[:, m, 0], 0.0)
            nc.vector.memset(P[:, m, Dp - 1], 0.0)
            nc.vector.memset(P[:, m, 1:Dp - 1, 0, :], 0.0)
            nc.vector.memset(P[:, m, 1:Dp - 1, Hp - 1, :], 0.0)
            nc.vector.memset(P[:, m, 1:Dp - 1, 1:Hp - 1, 0:1], 0.0)
            nc.vector.memset(P[:, m, 1:Dp - 1, 1:Hp - 1, Wp - 1:Wp], 0.0)

        # ---------------- conv1 ----------------
        # Column-tiled 2x: tile A (rows 0..27 of a half-plane) computes on
        # PSUM partitions 0-63, tile B (rows 28..55) on partitions 64-127,
        # so DVE stats/pool run with all 128 lanes.
        def ar_payload(stats_t, parts, sfx):
            """bn_aggr -> (mean, E[x^2]) payload -> AllReduce; returns
            the SBUF tile holding the summed payload."""
            mv = small.tile([parts, 1, 2], F32, tag=f"mv{sfx}")
            nc.vector.bn_aggr(mv[:, 0], stats_t[:, 0])
            cc = small.tile([parts, 2], F32, tag=f"cc{sfx}")
            sq = small.tile([parts, 1], F32, tag=f"sq{sfx}")
            nc.vector.tensor_mul(sq[:], mv[:, 0, 0:1], mv[:, 0, 0:1])
            nc.vector.tensor_add(cc[:, 1:2], mv[:, 0, 1:2], sq[:])
            nc.vector.tensor_copy(cc[:, 0:1], mv[:, 0, 0:1])
            ccin = dram.tile([parts, 2], F32, tag=f"ccin{sfx}")
            ccout = dram.tile([parts, 2], F32, tag=f"ccout{sfx}",
                              addr_space="Shared")
            nc.scalar.dma_start(ccin[:], cc[:])
            if fake_cc:
                nc.scalar.dma_start(ccout[:], ccin[:])
            else:
                nc.gpsimd.collective_compute(
                    "AllReduce", ADD, replica_groups=rg,
                    ins=[ccin.opt()], outs=[ccout.opt()])
            ar = small.tile([parts, 2], F32, tag=f"arx{sfx}")
            nc.scalar.dma_start(ar[:], ccout[:])
            return ar

        def bn_finish(name, mg_e2, parts, sfx):
            """mg_e2 [parts, 2] = (mean, E[x^2]) -> (s, t) tiles."""
            mg = mg_e2[:, 0:1]
            sq2 = small.tile([parts, 1], F32, tag=f"sq2{sfx}")
            nc.vector.tensor_mul(sq2[:], mg, mg)
            varg = small.tile([parts, 1], F32, tag=f"varg{sfx}")
            nc.vector.tensor_sub(varg[:], mg_e2[:, 1:2], sq2[:])
            sd = small.tile([parts, 1], F32, tag=f"sd{sfx}")
            nc.scalar.activation(sd[:], varg[:], SQRT, bias=eps_t[:parts])
            inv = small.tile([parts, 1], F32, tag=f"inv{sfx}")
            nc.vector.reciprocal(inv[:], sd[:])
            gt, bt = params[name]
            s_t = small.tile([parts, 1], F32, tag=f"s_{sfx}")
            t_t = small.tile([parts, 1], F32, tag=f"t_{sfx}")
            nc.vector.tensor_mul(s_t[:], inv[:], gt[:])
            tmn = small.tile([parts, 1], F32, tag=f"tmn{sfx}")
            nc.vector.tensor_mul(tmn[:], mg, s_t[:])
            nc.vector.tensor_sub(t_t[:], bt[:], tmn[:])
            return s_t, t_t

        def bn_reduce_conv1(arA, arB, fa, fb):
            """Combine two weighted AR payloads + fold column halves."""
            aU = small.tile([64, 2], F32, tag="arAU")
            nc.scalar.dma_start(aU[:], arA[64:128, :])
            bU = small.tile([64, 2], F32, tag="arBU")
            nc.scalar.dma_start(bU[:], arB[64:128, :])
            sA = small.tile([64, 2], F32, tag="sA")
            nc.vector.tensor_add(sA[:], arA[0:64, :], aU[:])
            sB = small.tile([64, 2], F32, tag="sB")
            nc.vector.tensor_add(sB[:], arB[0:64, :], bU[:])
            sAw = small.tile([64, 2], F32, tag="sAw")
            nc.vector.tensor_scalar_mul(sAw[:], sA[:], fa)
            sBw = small.tile([64, 2], F32, tag="sBw")
            nc.vector.tensor_scalar_mul(sBw[:], sB[:], fb)
            tot = small.tile([64, 2], F32, tag="tot1")
            nc.vector.tensor_add(tot[:], sAw[:], sBw[:])
            return bn_finish("1", tot, 64, "c1")

        with tc.tile_pool(name="x1p", bufs=3) as x1p, \
             tc.tile_pool(name="zplp", bufs=2) as zplp, \
             tc.tile_pool(name="ybfp", bufs=3) as ybfp, \
             tc.tile_pool(name="st1p", bufs=1) as st1p:
            stats1a = st1p.tile([128, 1, 210, 6], F32, tag="stats1a")
            stats1b = st1p.tile([128, 1, 14, 6], F32, tag="stats1b")
            ar1A = None
            with nc.named_scope("conv1"):
                for z in range(16):
                    for h in range(2):
                        slab = x1p.tile([81, 6272], BF16, tag="slab")
                        eng = nc.sync if (h == 0) else nc.scalar
                        eng.dma_start(
                            slab[:],
                            din["x1"][:, z, h * 6272:(h + 1) * 6272])
                        zpl = zplp.tile([128, 14, 56], BF16, tag="zpl")
                        for (i0, P2) in [(0, 2), (2, 2), (4, 2), (6, 1)]:
                            pst = psum.tile([128, 2, 512], F32, tag="ps",
                                            name="ps")
                            for k in range(P2):
                                i = i0 + k
                                nc.tensor.matmul(
                                    pst[0:64, k, :448], w1_sb[:],
                                    slab[:, i * 448:(i + 1) * 448],
                                    start=True, stop=True,
                                    tile_position=(0, 0))
                                nc.tensor.matmul(
                                    pst[64:128, k, :448], w1_sb[:],
                                    slab[:, (i + 7) * 448:(i + 8) * 448],
                                    start=True, stop=True,
                                    tile_position=(0, 64))
                            ybf = ybfp.tile([128, 2, 448], BF16, tag="ybf")
                            nc.scalar.activation(ybf[:, :P2],
                                                 pst[:, :P2, :448], COPY)
                            ti0 = z * 14 + h * 7 + i0
                            for k in range(P2):
                                ti = ti0 + k
                                dst = (stats1a[:, 0, ti] if ti < 210
                                       else stats1b[:, 0, ti - 210])
                                nc.vector.bn_stats(dst, ybf[:, k])
                            v = ybf[:, :P2].rearrange(
                                "p g (n t) -> p (g n) t", t=2)
                            pw = pwp.tile([128, 2, 4, 56], BF16, tag="pw")
                            nc.vector.tensor_reduce(
                                pw[:, :P2].rearrange(
                                    "p g a b -> p (g a b)"),
                                v, axis=AXX, op=MAX)
                            zv = zpl[:, 2 * i0:2 * i0 + 2 * P2, :].rearrange(
                                "p (g r) b -> p g r b", g=P2)
                            nc.vector.tensor_max(zv, pw[:, :P2, 0::2, :],
                                                 pw[:, :P2, 1::2, :])
                        r0 = 28 * h
                        nc.scalar.dma_start(
                            y1_dram[:, z, r0 * 56:(r0 + 14) * 56],
                            zpl[0:64].rearrange("p a b -> p (a b)"))
                        nc.scalar.dma_start(
                            y1_dram[:, z, (r0 + 14) * 56:(r0 + 28) * 56],
                            zpl[64:128].rearrange("p a b -> p (a b)"))
                    if z == 14:
                        # bulk-stats AllReduce overlaps the last z-plane
                        ar1A = ar_payload(stats1a, 128, "c1a")
            with nc.named_scope("ar1"):
                ar1B = ar_payload(stats1b, 128, "c1b")
                s1, t1 = bn_reduce_conv1(ar1A, ar1B,
                                         210.0 / 224 / 16, 14.0 / 224 / 16)

        # prefetch conv3a + conv3b weights early (scalar queue)
        w3a_sb = wpool.tile([128, 2, 27, 128], BF16, tag="w")
        nc.scalar.dma_start(
            w3a_sb[:].rearrange("p m t c -> p (m t c)"),
            din["w3a"][:].rearrange("p k m t c -> p (k m t c)"))
        w3b_sb = wpool.tile([128, 27, 2, 128], BF16, tag="w")
        nc.scalar.dma_start(
            w3b_sb[:].rearrange("p t j c -> p (t j c)"),
            din["w3b"][:].rearrange("p t j c -> p (t j c)"))

        # ---------------- conv2 ----------------
        stats2a = statsp.tile([128, 1, 105, 6], F32, tag="stats2a")
        stats2b = statsp.tile([128, 1, 7, 6], F32, tag="stats2b")
        ar2A = None
        P3in = arenaA.tile([128, 1, 10, 30, 30], BF16, tag="pin")
        zero_borders_m(P3in, 0, 10, 30, 30)
        with tc.tile_pool(name="plp", bufs=3) as plp, \
             tc.tile_pool(name="b2p", bufs=2) as b2p, \
             tc.tile_pool(name="c2p", bufs=2) as c2p, \
             tc.tile_pool(name="s2p", bufs=2) as s2p, \
             nc.named_scope("conv2"):

            def build_plane(dst64, pidx):
                if pidx == 0 or pidx == 17:
                    nc.vector.memset(dst64[:], 0.0)
                    return
                pl = plp.tile([64, 3136], BF16, tag="pl")
                nc.sync.dma_start(pl[:], y1_dram[:, pidx - 1, :])
                nc.vector.memset(dst64[:, 0, :], 0.0)
                nc.vector.memset(dst64[:, 57, :], 0.0)
                nc.vector.memset(dst64[:, 1:57, 0:1], 0.0)
                nc.vector.memset(dst64[:, 1:57, 57:58], 0.0)
                nc.scalar.activation(
                    dst64[:, 1:57, 1:57],
                    pl[:].rearrange("p (a b) -> p a b", a=56),
                    RELU, bias=t1[:, 0:1], scale=s1[:, 0:1])

            groups2 = [(0, 2), (2, 2), (4, 2), (6, 1)]
            s2_prev = None
            for z in range(16):
                B2 = b2p.tile([128, 58, 58], BF16, tag="b2")
                build_plane(B2[0:64], z)
                build_plane(B2[64:128], z + 1)
                C2 = c2p.tile([64, 58, 58], BF16, tag="c2")
                build_plane(C2[:], z + 2)
                S2z = s2p.tile([128, 28, 28], BF16, tag="s2z")
                for (t0, G) in groups2:
                    pst = psum.tile([128, 2, 512], F32, tag="ps", name="ps")
                    for k9 in range(9):
                        kh, kw = k9 // 3, k9 % 3
                        for j in range(G):
                            y0 = 8 * (t0 + j) + kh
                            nc.tensor.matmul(pst[:, j, :448],
                                             w2a_sb[:, k9, :],
                                             B2[:, y0:y0 + 8, kw:kw + 56],
                                             start=(k9 == 0), stop=False)
                    for k9 in range(9):
                        kh, kw = k9 // 3, k9 % 3
                        for j in range(G):
                            y0 = 8 * (t0 + j) + kh
                            nc.tensor.matmul(pst[:, j, :448],
                                             w2b_sb[:, k9, :],
                                             C2[:, y0:y0 + 8, kw:kw + 56],
                                             start=False, stop=(k9 == 8))
                    for j in range(G):
                        ti = z * 7 + t0 + j
                        dst = (stats2a[:, 0, ti] if ti < 105
                               else stats2b[:, 0, ti - 105])
                        nc.vector.bn_stats(dst, pst[:, j, :448])
                        v = pst[:, j, :448].rearrange(
                            "p (n t) -> p n t", t=2)
                        pw = pwp.tile([128, 8, 28], BF16, tag="pw")
                        nc.vector.tensor_reduce(
                            pw[:].rearrange("p a b -> p (a b)"),
                            v, axis=AXX, op=MAX)
                        ro = 4 * (t0 + j)
                        nc.vector.tensor_max(S2z[:, ro:ro + 4, :],
                                             pw[:, 0::2, :], pw[:, 1::2, :])
                if z % 2 == 1:
                    nc.vector.tensor_max(P3in[:, 0, 1 + z // 2, 1:29, 1:29],
                                         S2z[:], s2_prev[:])
                else:
                    s2_prev = S2z
                if z == 14:
                    ar2A = ar_payload(stats2a, 128, "c2a")
        with nc.named_scope("ar2"):
            ar2B = ar_payload(stats2b, 128, "c2b")
            sAw = small.tile([128, 2], F32, tag="sAw2")
            nc.vector.tensor_scalar_mul(sAw[:], ar2A[:], 105.0 / 112 / 8)
            sBw = small.tile([128, 2], F32, tag="sBw2")
            nc.vector.tensor_scalar_mul(sBw[:], ar2B[:], 7.0 / 112 / 8)
            tot2 = small.tile([128, 2], F32, tag="tot2")
            nc.vector.tensor_add(tot2[:], sAw[:], sBw[:])
            s2, t2 = bn_finish("2", tot2, 128, "c2")
        nc.scalar.activation(P3in[:, 0, 1:9, 1:29, 1:29],
                             P3in[:, 0, 1:9, 1:29, 1:29], RELU,
                             bias=t2[:, 0:1], scale=s2[:, 0:1])

        # ---------------- generic conv layers ----------------
        w_sbs = {"3a": w3a_sb, "3b": w3b_sb}
        nxt = {"3a": "3b", "3b": "4a", "4a": "4b", "4b": "5a",
               "5a": "5b", "5b": None}
        wshape = {"4a": [128, 27, 4, 128], "4b": [128, 27, 4, 128],
                  "5a": [128, 27, 4, 128], "5b": [128, 27, 4, 128]}

        def wslice(name, w_sb, m, c, tap):
            ti = tap[0] * 9 + tap[1] * 3 + tap[2]
            if name == "3a":
                return w_sb[:, m, ti, :]
            if name == "3b":
                return w_sb[:, ti, (m - c) % 2, :]
            if name == "4a":
                return w_sb[:, ti, (m - 2 * c) % 4, :]
            return w_sb[:, ti, (m - c) % 4, :]

        arena_out = {"3a": arenaB, "3b": arenaA, "4a": arenaB,
                     "4b": arenaA, "5a": arenaB}
        Pin = P3in
        for (name, Cin, Cout, D, H, W, pooled) in GEN_LAYERS:
            Kch, Mch = Cin // 128, Cout // 128
            tiles = layer_tiles(D, H, W)
            T = len(tiles)
            H2, W2, D2 = H // 2, W // 2, D // 2
            w_sb = w_sbs[name]
            # prefetch next layer's weights
            nn = nxt[name]
            if nn is not None and nn not in w_sbs:
                wn = wpool.tile(wshape[nn], BF16, tag="w")
                nc.scalar.dma_start(
                    wn[:].rearrange("p t j c -> p (t j c)"),
                    din[f"w{nn}"][:].rearrange("p t j c -> p (t j c)"))
                w_sbs[nn] = wn
            stats_t = statsp.tile([128, Mch, T, 6], F32, tag="stats")
            groups = [tiles[i:i + 2] for i in range(0, T, 2)]
            # AR granularity: per chunk when per-chunk compute >> AR
            # latency (4a/4b); bulk+last for short layers (5a/5b)
            parts_split = {
                "3a": [[0], [1]], "3b": [[0], [1]],
                "4a": [[0], [1], [2], [3]], "4b": [[0], [1], [2], [3]],
                "5a": [[0, 1, 2], [3]], "5b": [[0, 1, 2], [3]],
            }[name]

            if name != "5b":
                nD, nH, nW = (D2, H2, W2) if pooled else (D, H, W)
                Pnext = arena_out[name].tile(
                    [128, Mch, nD + 2, nH + 2, nW + 2], BF16, tag="pin")
                for mm in range(Mch):
                    zero_borders_m(Pnext, mm, nD + 2, nH + 2, nW + 2)
                out5b = None
            else:
                out5b = small.tile([128, 4, 2, 7, 7], BF16, tag="stage5b")
            st_parts = {}

            scope = nc.named_scope(f"conv{name}")
            scope.__enter__()
            for m in range(Mch):
                ti = 0
                for grp in groups:
                    G = len(grp)
                    pst = psum.tile([128, 2, 512], F32, tag="ps", name="ps")
                    nmm = Kch * 27
                    i = 0
                    for c in range(Kch):
                        for tap in TAPS:
                            kd, kh, kw = tap
                            lhs = wslice(name, w_sb, m, c, tap)
                            for j, (z0, y0, dz, dy) in enumerate(grp):
                                rhs = Pin[:, c, z0 + kd:z0 + kd + dz,
                                          y0 + kh:y0 + kh + dy,
                                          kw:kw + W]
                                nc.tensor.matmul(
                                    pst[:, j, :dz * dy * W], lhs, rhs,
                                    start=(i == 0), stop=(i == nmm - 1))
                            i += 1
                    for j, (z0, y0, dz, dy) in enumerate(grp):
                        N = dz * dy * W
                        nc.vector.bn_stats(stats_t[:, m, ti + j],
                                           pst[:, j, :N])
                        v = pst[:, j, :N].rearrange(
                            "p (a b c) -> p a b c", a=dz, b=dy)
                        if pooled is None:
                            nc.vector.tensor_copy(out5b[:, m], v)
                        elif pooled is False:
                            nc.vector.tensor_copy(
                                Pnext[:, m, 1 + z0:1 + z0 + dz,
                                      1 + y0:1 + y0 + dy, 1:1 + W], v)
                        else:
                            vx = pst[:, j, :N].rearrange(
                                "p (n t) -> p n t", t=2)
                            pw1 = pwp.tile([128, dz, dy, W2], BF16,
                                           tag="pw2", name="pw1")
                            nc.vector.tensor_reduce(
                                pw1[:].rearrange("p a b c -> p (a b c)"),
                                vx, axis=AXX, op=MAX)
                            pw2 = pwp.tile([128, dz, dy // 2, W2], BF16,
                                           tag="pw3", name="pw2")
                            nc.vector.tensor_max(pw2[:], pw1[:, :, 0::2, :],
                                                 pw1[:, :, 1::2, :])
                            nc.vector.tensor_max(
                                Pnext[:, m, 1 + z0 // 2:1 + (z0 + dz) // 2,
                                      1 + y0 // 2:1 + (y0 + dy) // 2,
                                      1:1 + W2],
                                pw2[:, 0::2, :, :], pw2[:, 1::2, :, :])
                    ti += G
                # stats reduce + apply per part
                for pi, mlist in enumerate(parts_split):
                    if m != mlist[-1]:
                        continue
                    with nc.named_scope(f"ar{name}_{pi}"):
                        s_t, t_t = bn_reduce_part(name, stats_t, 128,
                                                  mlist, T)
                    st_parts[pi] = (mlist, s_t, t_t)
                    if name == "5b":
                        continue
                    for ii, mm in enumerate(mlist):
                        iv = Pnext[:, mm, 1:1 + nD, 1:1 + nH, 1:1 + nW]
                        nc.scalar.activation(
                            iv, iv, RELU,
                            bias=t_t[:, ii:ii + 1], scale=s_t[:, ii:ii + 1])
            scope.__exit__(None, None, None)

            if name == "5b":
                # pool5: window (2,2,2) stride 2, pad (0,1,1)
                pd = small.tile([128, 4, 7, 7], BF16, tag="pd5")
                nc.vector.tensor_max(pd[:], out5b[:, :, 0], out5b[:, :, 1])
                pw5 = small.tile([128, 4, 7, 4], BF16, tag="pw5")
                nc.vector.tensor_copy(pw5[:, :, :, 0:1], pd[:, :, :, 0:1])
                nc.vector.tensor_max(pw5[:, :, :, 1:4],
                                     pd[:, :, :, 1::2], pd[:, :, :, 2::2])
                ph5 = small.tile([128, 4, 4, 4], BF16, tag="ph5")
                nc.vector.tensor_copy(ph5[:, :, 0:1, :], pw5[:, :, 0:1, :])
                nc.vector.tensor_max(ph5[:, :, 1:4, :],
                                     pw5[:, :, 1::2, :], pw5[:, :, 2::2, :])
                # BN+ReLU -> Z, then global mean (1/16 folded into fcw).
                # Per AR-part tiles so part-A FC matmuls run while the
                # last chunk's AllReduce is still in flight.
                feats = []
                for pi, (mlist, s_t, t_t) in st_parts.items():
                    nm = len(mlist)
                    Zp = small.tile([128, nm, 16], BF16, tag=f"z5_{pi}")
                    for ii, mm in enumerate(mlist):
                        nc.scalar.activation(
                            Zp[:, ii, :],
                            ph5[:, mm].rearrange("p a b -> p (a b)"),
                            RELU, bias=t_t[:, ii:ii + 1],
                            scale=s_t[:, ii:ii + 1])
                    fp = small.tile([128, nm], F32, tag=f"feat{pi}")
                    nc.vector.tensor_reduce(fp[:], Zp[:], axis=AXX, op=ADD)
                    fc_in = small.tile([128, nm], BF16, tag=f"fcin{pi}")
                    nc.vector.tensor_copy(fc_in[:], fp[:])
                    feats.append((mlist, fc_in))
                psf = psfc.tile([101, 1], F32, tag="psfc")
                ci = 0
                for mlist, fc_in in feats:
                    for ii, mm in enumerate(mlist):
                        nc.tensor.matmul(psf[:], fcw_sb[:, mm, :],
                                         fc_in[:, ii:ii + 1],
                                         start=(ci == 0), stop=(ci == 3))
                        ci += 1
                out_sb = small.tile([101, 1], F32, tag="outsb")
                nc.scalar.activation(out_sb[:], psf[:], IDENT,
                                     bias=fcb_sb[:])
                nc.sync.dma_start(logits[:], out_sb[:])
            else:
                Pin = Pnext


_STATE = {}


def _get_nc(n_cores=N_CORES):
    key = f"nc{n_cores}"
    if key not in _STATE:
        _STATE[key] = build_bass(n_cores)
    return _STATE[key]


def kernel(**inputs):
    nc = _get_nc()
    shared, x1_list = host_prep(inputs)
    in_maps = []
    for i in range(N_CORES):
        m = dict(shared)
        m["x1"] = x1_list[i]
        in_maps.append(m)
    res = run_bass_kernel_spmd(nc, in_maps, core_ids=list(range(N_CORES)))
    out = np.stack([res.results[i]["logits"].reshape(101)
                    for i in range(N_CORES)]).astype(np.float32)
    return out

